# revision 52
# baseline (speedup 1.0000x reference)
"""MeshUpdateNet (EdgeConv message passing + MLP decoder) on 8 Trainium2
NeuronCores via Bass/Tile.

Strategy (chosen over the edge-shard + all-reduce-max hint: sharding by
destination node needs no collectives at all):

  - Nodes are sharded by destination: sort nodes by degree (desc) and deal
    them round-robin to the 8 cores. Each core owns NC = N/8 nodes and all
    edges pointing at them (~E/8 per core, balanced), and its local node
    list is degree-sorted.
  - Edges are laid out rank-major: rank r holds the r-th edge of every
    local node with deg > r. Because nodes are degree-sorted, rank r's
    slots form a prefix [0, N_r) of the local node axis, so segment-max
    becomes a sequence of elementwise max ops over aligned prefixes - no
    scatter, no segmented reduce.
  - The host pre-gathers the per-slot features [xi ; xj] into a [6, L]
    bf16 stream per core (this is the sharding step: replicate+permute of
    x). The round-robin deal makes the rank widths common across cores
    (+-1, padded by duplicating an existing edge of the node - max is
    idempotent so duplicates are free), so one SPMD program serves all 8.
  - Device per core:  h1 = relu(w1m^T s + b1)  (PE K=6-pad-128 matmul + ACT)
                      h2 = w2^T h1             (PE K=128 matmul)
                      agg = max(agg, h2)       (DVE tensor_tensor, psum in)
    The stream is bound by the two PSUM evacuations (ACT relu-pass and
    DVE max-pass, both 1 elem/lane/cycle on TRN2 since matmul PSUM
    output must be fp32); measured DVE occupancy is ~99% in steady
    state, i.e. the stream runs at the hardware floor.
    Dense tail in 512-node tiles with w34 = w3@w4 folded on the host
    (no nonlinearity between them) and b3 folded into b4':
      r3 = relu(agg + b2) (DVE) -> r5 = relu(w34^T r3 + b4')
      (ACT/DVE alternating) -> dec = w5^T r5 packed 4 tiles per PSUM
      bank via tile_position col groups -> tanh (+b5) ->
      out = pos + 0.1*tanh (DVE scalar_tensor_tensor), all feature-major.
  - Scheduling: chunk DMAs are issued just-in-time inside the tile loop
    (an upfront prefetch chain serialized on Sync and stalled all
    engines ~82us); buffer zero-fills are placed per-engine so the
    in-order Vector queue stays clear of the stream's TT ops; tail
    constants are fetched mid-stream; each output group is DMA'd as
    soon as it completes; agg (bf16) is initialised to -1e30 via DMA
    from a host constant. Nodes with no edges are patched on the host
    with the closed-form constant output.
"""
import sys

sys.path.insert(0, '/opt/trn_rl_repo')

import numpy as np
import ml_dtypes

import concourse.bass as bass
import concourse.tile as tile
from concourse import bacc, mybir
from concourse import bass_utils

F32 = mybir.dt.float32
BF16 = mybir.dt.bfloat16
BF = ml_dtypes.bfloat16

N_CORES = 8
TILE_W = 1024      # edge tile width (2 psum banks)
MM_W = 512         # max matmul moving free dim
CHUNK = 8192       # stream DMA chunk (cols)
NODE_W = 512       # tail node-tile width
GROUP = 4          # node tiles packed per psum group in the tail
WARMUP_MM = 8      # gapless matmul chain to ramp the PE p-state


def make_schedule(deg, n_nodes):
    """Common (all-cores) edge/tail tiling from the global degree array."""
    nodes_sorted = np.argsort(-deg, kind='stable')
    deg_sorted = deg[nodes_sorted]
    d_max = int(deg_sorted[0]) if len(deg_sorted) else 0
    M = np.searchsorted(-deg_sorted, -(np.arange(d_max) + 1), side='right')
    NC = n_nodes // N_CORES
    N_r = -(-M // N_CORES)              # ceil(M_r/8): common rank width
    T_r = -(-N_r // TILE_W)
    L = int((T_r * TILE_W).sum())
    rank_off = np.zeros(d_max + 1, np.int64)
    np.cumsum(T_r * TILE_W, out=rank_off[1:])
    etiles = []
    for r in range(d_max):
        w_left = int(N_r[r])
        for t in range(int(T_r[r])):
            w = min(TILE_W, w_left - t * TILE_W)
            etiles.append((int(rank_off[r]) + t * TILE_W, t * TILE_W, w))
    n_ntiles = -(-NC // NODE_W)
    n_groups = -(-n_ntiles // GROUP)
    return dict(nodes_sorted=nodes_sorted, deg_sorted=deg_sorted, d_max=d_max,
                NC=NC, N_r=N_r, T_r=T_r, L=L, rank_off=rank_off, etiles=etiles,
                n_ntiles=n_ntiles, n_groups=n_groups)


def build_nc(sched):
    NC, L = sched['NC'], sched['L']
    etiles = sched['etiles']
    n_ntiles, n_groups = sched['n_ntiles'], sched['n_groups']
    GPC = n_groups * NODE_W
    n_chunks = -(-L // CHUNK)

    nc = bacc.Bacc("TRN2", target_bir_lowering=False, debug=False,
                   enable_asserts=False, num_devices=N_CORES)

    # chunk-grouped stream layout: chunk ci's 6 rows live at rows
    # [6ci, 6ci+6) and are contiguous in DRAM (better DMA locality)
    xs_d = nc.dram_tensor("xs", [6 * n_chunks, CHUNK], BF16,
                          kind="ExternalInput").ap()
    pospk_d = nc.dram_tensor("pospk", [99, GPC], F32, kind="ExternalInput").ap()
    w1m_d = nc.dram_tensor("w1m", [128, 128], BF16, kind="ExternalInput").ap()
    w2_d = nc.dram_tensor("w2", [128, 128], BF16, kind="ExternalInput").ap()
    # w34 = w3 @ w4 folded on the host (no nonlinearity between them)
    w34_d = nc.dram_tensor("w34", [128, 128], BF16, kind="ExternalInput").ap()
    w5_d = nc.dram_tensor("w5", [128, 3], BF16, kind="ExternalInput").ap()
    b1_d = nc.dram_tensor("b1", [128, 1], F32, kind="ExternalInput").ap()
    agg0_d = nc.dram_tensor("agg0", [128, NC], BF16, kind="ExternalInput").ap()
    b2_d = nc.dram_tensor("b2", [128, 1], F32, kind="ExternalInput").ap()
    b4p_d = nc.dram_tensor("b4p", [128, 1], F32, kind="ExternalInput").ap()
    b5pk_d = nc.dram_tensor("b5pk", [99, 1], F32, kind="ExternalInput").ap()
    out_d = nc.dram_tensor("outpk", [99, GPC], F32, kind="ExternalOutput").ap()

    RELU = mybir.ActivationFunctionType.Relu
    TANH = mybir.ActivationFunctionType.Tanh
    COPY = mybir.ActivationFunctionType.Copy
    ADD = mybir.AluOpType.add
    MAX = mybir.AluOpType.max
    MULT = mybir.AluOpType.mult

    NBUF = 4
    LOOKAHEAD = 2
    r0_end = int(sched['rank_off'][1])

    with tile.TileContext(nc) as tc:
        with (
            tc.tile_pool(name="const", bufs=1) as cp,
            tc.tile_pool(name="aggp", bufs=1) as aggp,
            tc.tile_pool(name="stream", bufs=1) as sp,
            tc.tile_pool(name="work", bufs=4) as wp,
        ):
            # constants needed early (first on the Sync queue)
            w2_s = cp.tile([128, 128], BF16)
            nc.sync.dma_start(w2_s[:], w2_d[:])
            w1m_s = cp.tile([128, 128], BF16)
            nc.sync.dma_start(w1m_s[:], w1m_d[:])
            b1_s = cp.tile([128, 1], F32)
            nc.sync.dma_start(b1_s[:], b1_d[:])

            warm_rhs = wp.tile([128, 512], BF16, tag="warmrhs")
            nc.vector.memset(warm_rhs[:], 0.0)

            # Stream buffers: rows 0-5 carry the DMA'd [xi;xj] stream;
            # rows 6-127 are zeroed once and never rewritten, so mm1
            # contracts over K=128 with a zero-padded w1m. buf0 on
            # Vector in quarters (fast path for the first chunk); the
            # rest on GpSimd so the Vector queue stays clear for the
            # stream's in-order TT ops.
            ch_bufs = []
            for bi in range(NBUF):
                chb = sp.tile([128, CHUNK], BF16, tag=f"xs{bi}",
                              name=f"chb{bi}")
                if bi == 0:
                    q = CHUNK // 4
                    for k in range(4):
                        nc.vector.memset(chb[:, k * q:(k + 1) * q], 0.0)
                else:
                    nc.gpsimd.memset(chb[:], 0.0)
                ch_bufs.append(chb)

            # just-in-time chunk DMA issuance (keeps the Sync queue free
            # of upfront cross-buffer waits that would gate the stream)
            issued = [-1]

            def ensure_chunk(ci):
                tgt = min(ci + LOOKAHEAD, n_chunks - 1)
                while issued[0] < tgt:
                    issued[0] += 1
                    c = issued[0]
                    ch = ch_bufs[c % NBUF]
                    # finer splits early: chunk 0's col-quarters pair
                    # with its memset quarters so the first tiles start
                    # as soon as possible
                    ncol = 4 if c == 0 else 2
                    nrow = 2 if c <= 2 else 1
                    w = CHUNK // ncol
                    for k in range(ncol):
                        for (r0, r1) in ([(0, 3), (3, 6)] if nrow == 2
                                         else [(0, 6)]):
                            nc.sync.dma_start(
                                ch[r0:r1, k * w:(k + 1) * w],
                                xs_d[6 * c + r0:6 * c + r1,
                                     k * w:(k + 1) * w])

            ensure_chunk(0)
            # agg init streamed from a host constant, in slices so the
            # first stream tiles only wait on the slice they touch
            agg = aggp.tile([128, NC], BF16)
            a3 = NC // 3
            nc.sync.dma_start(agg[:, :a3], agg0_d[:, :a3])
            nc.sync.dma_start(agg[:, a3:2 * a3], agg0_d[:, a3:2 * a3])
            nc.sync.dma_start(agg[:, 2 * a3:], agg0_d[:, 2 * a3:])

            # PE warm-up: gapless back-to-back matmul chain in its own
            # psum scope (4 slots so slot-release never stalls the chain);
            # the p-state ramp needs >3us of uninterrupted PE execution.
            # Runs while the first chunk DMAs land.
            with tc.tile_pool(name="psW", bufs=4, space="PSUM") as pW:
                for i in range(WARMUP_MM):
                    wps = pW.tile([128, 512], F32, tag="warm")
                    nc.tensor.matmul(wps[:], w2_s[:], warm_rhs[:],
                                     start=True, stop=True)

            with (
                tc.tile_pool(name="psA", bufs=2, space="PSUM") as pA,
                tc.tile_pool(name="psB", bufs=2, space="PSUM") as pB,
            ):
                # tail constants: issued mid-stream (Sync queue is sparse
                # there) so they're resident well before the tail starts
                tail_tiles = {}

                def issue_tail_consts():
                    w34_s = cp.tile([128, 128], BF16)
                    nc.sync.dma_start(w34_s[:], w34_d[:])
                    w5_s = cp.tile([128, 3], BF16)
                    nc.sync.dma_start(w5_s[:], w5_d[:])
                    b2_s = cp.tile([128, 1], F32)
                    nc.sync.dma_start(b2_s[:], b2_d[:])
                    b4p_s = cp.tile([128, 1], F32)
                    nc.sync.dma_start(b4p_s[:], b4p_d[:])
                    b5pk_s = cp.tile([99, 1], F32)
                    nc.sync.dma_start(b5pk_s[:], b5pk_d[:])
                    pospk_s = cp.tile([99, GPC], F32)
                    nc.sync.dma_start(pospk_s[:], pospk_d[:])
                    tail_tiles.update(w34_s=w34_s, w5_s=w5_s, b2_s=b2_s,
                                      b4p_s=b4p_s, b5pk_s=b5pk_s,
                                      pospk_s=pospk_s)

                for ti, (so, c0, W) in enumerate(etiles):
                    if ti == 120:
                        issue_tail_consts()
                    ci, off = so // CHUNK, so % CHUNK
                    ensure_chunk(ci)
                    ch = ch_bufs[ci % NBUF]
                    ps1 = pA.tile([128, TILE_W], F32, tag="p1")
                    for h in range(0, W, MM_W):
                        w = min(MM_W, W - h)
                        nc.tensor.matmul(ps1[:, h:h + w], w1m_s[:],
                                         ch[:, off + h: off + h + w],
                                         start=True, stop=True)
                    h1 = wp.tile([128, TILE_W], BF16, tag="h1")
                    nc.scalar.activation(h1[:, :W], ps1[:, :W], RELU,
                                         bias=b1_s[:, 0:1])
                    ps2 = pB.tile([128, TILE_W], F32, tag="p2")
                    for h in range(0, W, MM_W):
                        w = min(MM_W, W - h)
                        nc.tensor.matmul(ps2[:, h:h + w], w2_s[:],
                                         h1[:, h:h + w], start=True, stop=True)
                    nc.vector.tensor_tensor(
                        out=agg[:, c0:c0 + W], in0=ps2[:, :W],
                        in1=agg[:, c0:c0 + W], op=MAX)

            w34_s = tail_tiles['w34_s']
            w5_s = tail_tiles['w5_s']
            b2_s = tail_tiles['b2_s']
            b4p_s = tail_tiles['b4p_s']
            b5pk_s = tail_tiles['b5pk_s']
            pospk_s = tail_tiles['pospk_s']
            outpk_s = cp.tile([99, GPC], F32)

            with (
                tc.tile_pool(name="psT", bufs=4, space="PSUM") as pT,
                tc.tile_pool(name="psG", bufs=2, space="PSUM") as pG,
            ):
                for g in range(n_groups):
                    ps5 = pG.tile([99, NODE_W], F32, tag="p5")
                    nc.vector.memset(ps5[:], 0.0)
                    for j in range(GROUP):
                        t = g * GROUP + j
                        if t >= n_ntiles:
                            break
                        c0 = t * NODE_W
                        W = min(NODE_W, NC - c0)
                        # r3 = relu(agg + b2) -> bf16, on DVE (4x mode)
                        r3 = wp.tile([128, NODE_W], BF16, tag="r3")
                        nc.vector.tensor_scalar(
                            out=r3[:, :W], in0=agg[:, c0:c0 + W],
                            scalar1=b2_s[:, 0:1], scalar2=0.0,
                            op0=ADD, op1=MAX)
                        ps4 = pT.tile([128, NODE_W], F32, tag="p4")
                        nc.tensor.matmul(ps4[:, :W], w34_s[:], r3[:, :W],
                                         start=True, stop=True)
                        r5 = wp.tile([128, NODE_W], BF16, tag="r5")
                        if j in (1, 3):
                            nc.vector.tensor_scalar(
                                out=r5[:, :W], in0=ps4[:, :W],
                                scalar1=b4p_s[:, 0:1], scalar2=0.0,
                                op0=ADD, op1=MAX)
                        else:
                            nc.scalar.activation(r5[:, :W], ps4[:, :W], RELU,
                                                 bias=b4p_s[:, 0:1])
                        nc.tensor.matmul(ps5[32 * j:32 * j + 3, :W], w5_s[:],
                                         r5[:, :W], start=True, stop=True,
                                         tile_position=(0, 32 * j))
                    s_t = wp.tile([99, NODE_W], F32, tag="s")
                    nc.scalar.activation(s_t[:], ps5[:], TANH,
                                         bias=b5pk_s[:, 0:1])
                    gc = g * NODE_W
                    nc.vector.scalar_tensor_tensor(
                        out=outpk_s[:, gc:gc + NODE_W], in0=s_t[:],
                        scalar=0.1, in1=pospk_s[:, gc:gc + NODE_W],
                        op0=MULT, op1=ADD)
                    # stream each group's slab out as soon as it's ready
                    nc.sync.dma_start(out_d[:, gc:gc + NODE_W],
                                      outpk_s[:, gc:gc + NODE_W])
    nc.compile()
    return nc


def make_inputs(x, pos, w1, b1, w2, b2, w3, b3, w4, b4, w5, b5,
                src, dst, sched):
    n_nodes = x.shape[0]
    E = src.shape[0]
    NC, L, d_max = sched['NC'], sched['L'], sched['d_max']
    N_r, rank_off = sched['N_r'], sched['rank_off']
    nodes_sorted = sched['nodes_sorted']
    n_groups = sched['n_groups']
    GPC = n_groups * NODE_W

    order = np.argsort(dst, kind='stable')
    src_sorted = src[order]
    deg = np.bincount(dst, minlength=n_nodes)
    starts = np.zeros(n_nodes + 1, np.int64)
    np.cumsum(deg, out=starts[1:])

    # msg @ w1 = [xi ; xj-xi] @ w1 = [xi ; xj] @ [[w1a-w1b]; [w1b]]
    w1a, w1b = w1[:3], w1[3:]
    w1m = np.zeros((128, 128), np.float32)
    w1m[:6] = np.vstack([w1a - w1b, w1b])
    w1m = w1m.astype(BF)
    b4p = (b3 @ w4 + b4).astype(np.float32).reshape(128, 1)   # fold b3
    b5pk = np.zeros((99, 1), np.float32)
    for j in range(GROUP):
        b5pk[32 * j:32 * j + 3, 0] = b5

    common = dict(
        w1m=w1m, w2=w2.astype(BF), w34=(w3 @ w4).astype(BF),
        w5=w5.astype(BF), b1=b1.reshape(128, 1).astype(np.float32),
        b2=b2.reshape(128, 1).astype(np.float32), b4p=b4p, b5pk=b5pk,
        agg0=np.full((128, NC), -1e30, BF))

    slot_pos = np.zeros(L, np.int64)
    for r in range(d_max):
        w = int(N_r[r])
        o = int(rank_off[r])
        slot_pos[o:o + w] = np.arange(w)

    in_maps = []
    for c in range(N_CORES):
        loc_nodes = nodes_sorted[c::N_CORES]
        loc_deg = deg[loc_nodes]
        loc_start = starts[loc_nodes]
        slot_src = np.zeros(L, np.int64)
        for r in range(d_max):
            w = int(N_r[r])
            o = int(rank_off[r])
            has = loc_deg[:w] > r
            # pad slots duplicate the node's first edge (max-idempotent);
            # deg-0 nodes gather garbage and are patched on the host
            idx = np.where(has, loc_start[:w] + r, loc_start[:w])
            np.minimum(idx, E - 1, out=idx)
            slot_src[o:o + w] = src_sorted[idx]
        xi_loc = x[loc_nodes]
        n_chunks = -(-L // CHUNK)
        Lp = n_chunks * CHUNK
        xs_flat = np.zeros((6, Lp), BF)
        xs_flat[0:3, :L] = xi_loc[slot_pos].T.astype(BF)
        xs_flat[3:6, :L] = x[slot_src].T.astype(BF)
        # chunk-grouped layout: [n_chunks, 6, CHUNK] -> [6*n_chunks, CHUNK]
        xs = np.ascontiguousarray(
            xs_flat.reshape(6, n_chunks, CHUNK).transpose(1, 0, 2)
        ).reshape(6 * n_chunks, CHUNK)
        # pack pos tiles 4-per-group into partition strips 32j..32j+2
        pos_t = np.zeros((3, n_groups * GROUP * NODE_W), np.float32)
        pos_t[:, :NC] = pos[loc_nodes].T
        ptiles = pos_t.reshape(3, n_groups * GROUP, NODE_W)
        pospk = np.zeros((99, n_groups, NODE_W), np.float32)
        for j in range(GROUP):
            pospk[32 * j:32 * j + 3] = ptiles[:, j::GROUP, :]
        in_maps.append(dict(xs=xs, pospk=pospk.reshape(99, GPC), **common))
    return in_maps


def unpack_outputs(results, sched, pos, deg, w3, b3, w4, b4, w5, b5):
    NC = sched['NC']
    nodes_sorted = sched['nodes_sorted']
    n_groups = sched['n_groups']
    n = len(nodes_sorted)
    out_full = np.zeros((n, 3), np.float32)
    for c in range(N_CORES):
        outpk = results[c]['outpk'].reshape(99, n_groups, NODE_W)
        tiles = np.zeros((3, n_groups * GROUP, NODE_W), np.float32)
        for j in range(GROUP):
            tiles[:, j::GROUP, :] = outpk[32 * j:32 * j + 3]
        out_t = tiles.reshape(3, -1)[:, :NC]
        out_full[nodes_sorted[c::N_CORES]] = out_t.T
    deg0 = deg == 0
    if deg0.any():
        # closed form for isolated nodes: agg = 0 -> enc = b3
        enc0 = b3
        dec0 = np.maximum(enc0 @ w4 + b4, 0.0) @ w5 + b5
        out_full[deg0] = pos[deg0] + 0.1 * np.tanh(dec0)
    return out_full


def run(inputs, trace=False, tmpdir=None):
    x = np.asarray(inputs['x'], np.float32)
    pos = np.asarray(inputs['pos'], np.float32)
    ei = np.asarray(inputs['edge_index'])
    src = ei[0].astype(np.int64)
    dst = ei[1].astype(np.int64)
    deg = np.bincount(dst, minlength=x.shape[0])
    sched = make_schedule(deg, x.shape[0])
    nc = build_nc(sched)
    args = [np.asarray(inputs[k], np.float32) for k in
            ('w1', 'b1', 'w2', 'b2', 'w3', 'b3', 'w4', 'b4', 'w5', 'b5')]
    in_maps = make_inputs(x, pos, *args, src, dst, sched)
    res = bass_utils.run_bass_kernel_spmd(
        nc, in_maps, core_ids=list(range(N_CORES)), trace=trace, tmpdir=tmpdir)
    w3_, b3_, w4_, b4_, w5_, b5_ = args[4:]
    out = unpack_outputs(res.results, sched, pos, deg,
                         w3_, b3_, w4_, b4_, w5_, b5_)
    return out, res


def kernel(**inputs):
    out, _ = run(inputs, trace=False)
    return out



# revision 54
# speedup vs baseline: 1.1720x; 1.1720x over previous
"""MeshUpdateNet (EdgeConv message passing + MLP decoder) on 8 Trainium2
NeuronCores via Bass/Tile.

Strategy (chosen over the edge-shard + all-reduce-max hint: sharding by
destination node needs no collectives at all):

  - Nodes are sharded by destination: sort nodes by degree (desc) and deal
    them round-robin to the 8 cores. Each core owns NC = N/8 nodes and all
    edges pointing at them (~E/8 per core, balanced), and its local node
    list is degree-sorted.
  - Edges are laid out rank-major: rank r holds the r-th edge of every
    local node with deg > r. Because nodes are degree-sorted, rank r's
    slots form a prefix [0, N_r) of the local node axis, so segment-max
    becomes a sequence of elementwise max ops over aligned prefixes - no
    scatter, no segmented reduce.
  - The host pre-gathers the per-slot features [xi ; xj] into a [6, L]
    bf16 stream per core (this is the sharding step: replicate+permute of
    x). The round-robin deal makes the rank widths common across cores
    (+-1, padded by duplicating an existing edge of the node - max is
    idempotent so duplicates are free), so one SPMD program serves all 8.
  - Device per core:  h1 = relu(w1m^T s + b1)  (PE K=6-pad-128 matmul + ACT)
                      h2 = w2^T h1             (PE K=128 matmul)
                      agg = max(agg, h2)       (DVE tensor_tensor, psum in)
    The stream is bound by the two PSUM evacuations (ACT relu-pass and
    DVE max-pass, both 1 elem/lane/cycle on TRN2 since matmul PSUM
    output must be fp32); measured DVE occupancy is ~99% in steady
    state, i.e. the stream runs at the hardware floor.
    Dense tail in 512-node tiles with w34 = w3@w4 folded on the host
    (no nonlinearity between them) and b3 folded into b4':
      r3 = relu(agg + b2) (DVE) -> r5 = relu(w34^T r3 + b4')
      (ACT/DVE alternating) -> dec = w5^T r5 packed 4 tiles per PSUM
      bank via tile_position col groups -> tanh (+b5) ->
      out = pos + 0.1*tanh (DVE scalar_tensor_tensor), all feature-major.
  - Scheduling: chunk DMAs are issued just-in-time inside the tile loop
    (an upfront prefetch chain serialized on Sync and stalled all
    engines ~82us); buffer zero-fills are placed per-engine so the
    in-order Vector queue stays clear of the stream's TT ops; tail
    constants are fetched mid-stream; each output group is DMA'd as
    soon as it completes; agg (bf16) is initialised to -1e30 via DMA
    from a host constant. Nodes with no edges are patched on the host
    with the closed-form constant output.
"""
import sys

sys.path.insert(0, '/opt/trn_rl_repo')

import numpy as np
import ml_dtypes

import concourse.bass as bass
import concourse.tile as tile
from concourse import bacc, mybir
from concourse import bass_utils

F32 = mybir.dt.float32
BF16 = mybir.dt.bfloat16
BF = ml_dtypes.bfloat16

N_CORES = 8
TILE_W = 1024      # edge tile width (2 psum banks)
MM_W = 512         # max matmul moving free dim
CHUNK = 8192       # stream DMA chunk (cols)
NODE_W = 512       # tail node-tile width
GROUP = 4          # node tiles packed per psum group in the tail
WARMUP_MM = 10     # gapless matmul chain to ramp the PE p-state


def make_schedule(deg, n_nodes):
    """Common (all-cores) edge/tail tiling from the global degree array."""
    nodes_sorted = np.argsort(-deg, kind='stable')
    deg_sorted = deg[nodes_sorted]
    d_max = int(deg_sorted[0]) if len(deg_sorted) else 0
    M = np.searchsorted(-deg_sorted, -(np.arange(d_max) + 1), side='right')
    NC = n_nodes // N_CORES
    N_r = -(-M // N_CORES)              # ceil(M_r/8): common rank width
    T_r = -(-N_r // TILE_W)
    L = int((T_r * TILE_W).sum())
    rank_off = np.zeros(d_max + 1, np.int64)
    np.cumsum(T_r * TILE_W, out=rank_off[1:])
    etiles = []
    for r in range(d_max):
        w_left = int(N_r[r])
        for t in range(int(T_r[r])):
            w = min(TILE_W, w_left - t * TILE_W)
            etiles.append((int(rank_off[r]) + t * TILE_W, t * TILE_W, w))
    n_ntiles = -(-NC // NODE_W)
    n_groups = -(-n_ntiles // GROUP)
    return dict(nodes_sorted=nodes_sorted, deg_sorted=deg_sorted, d_max=d_max,
                NC=NC, N_r=N_r, T_r=T_r, L=L, rank_off=rank_off, etiles=etiles,
                n_ntiles=n_ntiles, n_groups=n_groups)


def build_nc(sched):
    NC, L = sched['NC'], sched['L']
    etiles = sched['etiles']
    n_ntiles, n_groups = sched['n_ntiles'], sched['n_groups']
    GPC = n_groups * NODE_W
    n_chunks = -(-L // CHUNK)

    nc = bacc.Bacc("TRN2", target_bir_lowering=False, debug=False,
                   enable_asserts=False, num_devices=N_CORES)

    # chunk-grouped stream layout: chunk ci's 6 rows live at rows
    # [6ci, 6ci+6) and are contiguous in DRAM (better DMA locality)
    xs_d = nc.dram_tensor("xs", [6 * n_chunks, CHUNK], BF16,
                          kind="ExternalInput").ap()
    pospk_d = nc.dram_tensor("pospk", [99, GPC], F32, kind="ExternalInput").ap()
    w1m_d = nc.dram_tensor("w1m", [128, 128], BF16, kind="ExternalInput").ap()
    w2_d = nc.dram_tensor("w2", [128, 128], BF16, kind="ExternalInput").ap()
    # w34 = w3 @ w4 folded on the host (no nonlinearity between them)
    w34_d = nc.dram_tensor("w34", [128, 128], BF16, kind="ExternalInput").ap()
    w5_d = nc.dram_tensor("w5", [128, 3], BF16, kind="ExternalInput").ap()
    b1_d = nc.dram_tensor("b1", [128, 1], F32, kind="ExternalInput").ap()
    agg0_d = nc.dram_tensor("agg0", [128, NC], BF16, kind="ExternalInput").ap()
    b2_d = nc.dram_tensor("b2", [128, 1], F32, kind="ExternalInput").ap()
    b4p_d = nc.dram_tensor("b4p", [128, 1], F32, kind="ExternalInput").ap()
    b5pk_d = nc.dram_tensor("b5pk", [99, 1], F32, kind="ExternalInput").ap()
    out_d = nc.dram_tensor("outpk", [99, GPC], F32, kind="ExternalOutput").ap()

    RELU = mybir.ActivationFunctionType.Relu
    TANH = mybir.ActivationFunctionType.Tanh
    COPY = mybir.ActivationFunctionType.Copy
    ADD = mybir.AluOpType.add
    MAX = mybir.AluOpType.max
    MULT = mybir.AluOpType.mult

    NBUF = 4
    LOOKAHEAD = 2
    r0_end = int(sched['rank_off'][1])

    with tile.TileContext(nc) as tc:
        with (
            tc.tile_pool(name="const", bufs=1) as cp,
            tc.tile_pool(name="aggp", bufs=1) as aggp,
            tc.tile_pool(name="stream", bufs=1) as sp,
            tc.tile_pool(name="work", bufs=4) as wp,
        ):
            # constants needed early (first on the Sync queue)
            w2_s = cp.tile([128, 128], BF16)
            nc.sync.dma_start(w2_s[:], w2_d[:])
            w1m_s = cp.tile([128, 128], BF16)
            nc.sync.dma_start(w1m_s[:], w1m_d[:])
            b1_s = cp.tile([128, 1], F32)
            nc.sync.dma_start(b1_s[:], b1_d[:])

            warm_rhs = wp.tile([128, 512], BF16, tag="warmrhs")
            nc.vector.memset(warm_rhs[:], 0.0)

            # Stream buffers: rows 0-5 carry the DMA'd [xi;xj] stream;
            # rows 6-127 are zeroed once and never rewritten, so mm1
            # contracts over K=128 with a zero-padded w1m. buf0 on
            # Vector in quarters (fast path for the first chunk); the
            # rest on GpSimd so the Vector queue stays clear for the
            # stream's in-order TT ops.
            ch_bufs = []
            for bi in range(NBUF):
                chb = sp.tile([128, CHUNK], BF16, tag=f"xs{bi}",
                              name=f"chb{bi}")
                if bi == 0:
                    q = CHUNK // 4
                    for k in range(4):
                        nc.vector.memset(chb[:, k * q:(k + 1) * q], 0.0)
                else:
                    nc.gpsimd.memset(chb[:], 0.0)
                ch_bufs.append(chb)

            # just-in-time chunk DMA issuance (keeps the Sync queue free
            # of upfront cross-buffer waits that would gate the stream)
            issued = [-1]

            def ensure_chunk(ci):
                tgt = min(ci + LOOKAHEAD, n_chunks - 1)
                while issued[0] < tgt:
                    issued[0] += 1
                    c = issued[0]
                    ch = ch_bufs[c % NBUF]
                    # finer splits early: chunk 0's col-quarters pair
                    # with its memset quarters so the first tiles start
                    # as soon as possible
                    ncol = 4 if c == 0 else 2
                    nrow = 2 if c <= 2 else 1
                    w = CHUNK // ncol
                    for k in range(ncol):
                        for (r0, r1) in ([(0, 3), (3, 6)] if nrow == 2
                                         else [(0, 6)]):
                            nc.sync.dma_start(
                                ch[r0:r1, k * w:(k + 1) * w],
                                xs_d[6 * c + r0:6 * c + r1,
                                     k * w:(k + 1) * w])

            ensure_chunk(0)
            # agg init streamed from a host constant, in slices so the
            # first stream tiles only wait on the slice they touch
            agg = aggp.tile([128, NC], BF16)
            a3 = NC // 3
            nc.sync.dma_start(agg[:, :a3], agg0_d[:, :a3])
            nc.sync.dma_start(agg[:, a3:2 * a3], agg0_d[:, a3:2 * a3])
            nc.sync.dma_start(agg[:, 2 * a3:], agg0_d[:, 2 * a3:])

            # PE warm-up: gapless back-to-back matmul chain in its own
            # psum scope (4 slots so slot-release never stalls the chain);
            # the p-state ramp needs >3us of uninterrupted PE execution.
            # Runs while the first chunk DMAs land.
            with tc.tile_pool(name="psW", bufs=4, space="PSUM") as pW:
                for i in range(WARMUP_MM):
                    wps = pW.tile([128, 512], F32, tag="warm")
                    nc.tensor.matmul(wps[:], w2_s[:], warm_rhs[:],
                                     start=True, stop=True)

            with (
                tc.tile_pool(name="psA", bufs=2, space="PSUM") as pA,
                tc.tile_pool(name="psB", bufs=2, space="PSUM") as pB,
            ):
                # tail constants: issued mid-stream (Sync queue is sparse
                # there) so they're resident well before the tail starts
                tail_tiles = {}

                def issue_tail_consts():
                    w34_s = cp.tile([128, 128], BF16)
                    nc.sync.dma_start(w34_s[:], w34_d[:])
                    w5_s = cp.tile([128, 3], BF16)
                    nc.sync.dma_start(w5_s[:], w5_d[:])
                    b2_s = cp.tile([128, 1], F32)
                    nc.sync.dma_start(b2_s[:], b2_d[:])
                    b4p_s = cp.tile([128, 1], F32)
                    nc.sync.dma_start(b4p_s[:], b4p_d[:])
                    b5pk_s = cp.tile([99, 1], F32)
                    nc.sync.dma_start(b5pk_s[:], b5pk_d[:])
                    pospk_s = cp.tile([99, GPC], F32)
                    nc.sync.dma_start(pospk_s[:], pospk_d[:])
                    tail_tiles.update(w34_s=w34_s, w5_s=w5_s, b2_s=b2_s,
                                      b4p_s=b4p_s, b5pk_s=b5pk_s,
                                      pospk_s=pospk_s)

                for ti, (so, c0, W) in enumerate(etiles):
                    if ti == 120:
                        issue_tail_consts()
                    ci, off = so // CHUNK, so % CHUNK
                    ensure_chunk(ci)
                    ch = ch_bufs[ci % NBUF]
                    ps1 = pA.tile([128, TILE_W], F32, tag="p1")
                    for h in range(0, W, MM_W):
                        w = min(MM_W, W - h)
                        nc.tensor.matmul(ps1[:, h:h + w], w1m_s[:],
                                         ch[:, off + h: off + h + w],
                                         start=True, stop=True)
                    h1 = wp.tile([128, TILE_W], BF16, tag="h1")
                    nc.scalar.activation(h1[:, :W], ps1[:, :W], RELU,
                                         bias=b1_s[:, 0:1])
                    ps2 = pB.tile([128, TILE_W], F32, tag="p2")
                    for h in range(0, W, MM_W):
                        w = min(MM_W, W - h)
                        nc.tensor.matmul(ps2[:, h:h + w], w2_s[:],
                                         h1[:, h:h + w], start=True, stop=True)
                    nc.vector.tensor_tensor(
                        out=agg[:, c0:c0 + W], in0=ps2[:, :W],
                        in1=agg[:, c0:c0 + W], op=MAX)

            w34_s = tail_tiles['w34_s']
            w5_s = tail_tiles['w5_s']
            b2_s = tail_tiles['b2_s']
            b4p_s = tail_tiles['b4p_s']
            b5pk_s = tail_tiles['b5pk_s']
            pospk_s = tail_tiles['pospk_s']
            outpk_s = cp.tile([99, GPC], F32)

            with (
                tc.tile_pool(name="psT", bufs=4, space="PSUM") as pT,
                tc.tile_pool(name="psG", bufs=2, space="PSUM") as pG,
            ):
                for g in range(n_groups):
                    ps5 = pG.tile([99, NODE_W], F32, tag="p5")
                    nc.vector.memset(ps5[:], 0.0)
                    for j in range(GROUP):
                        t = g * GROUP + j
                        if t >= n_ntiles:
                            break
                        c0 = t * NODE_W
                        W = min(NODE_W, NC - c0)
                        # r3 = relu(agg + b2) -> bf16, on DVE (4x mode)
                        r3 = wp.tile([128, NODE_W], BF16, tag="r3")
                        nc.vector.tensor_scalar(
                            out=r3[:, :W], in0=agg[:, c0:c0 + W],
                            scalar1=b2_s[:, 0:1], scalar2=0.0,
                            op0=ADD, op1=MAX)
                        ps4 = pT.tile([128, NODE_W], F32, tag="p4")
                        nc.tensor.matmul(ps4[:, :W], w34_s[:], r3[:, :W],
                                         start=True, stop=True)
                        r5 = wp.tile([128, NODE_W], BF16, tag="r5")
                        if j in (1, 3):
                            nc.vector.tensor_scalar(
                                out=r5[:, :W], in0=ps4[:, :W],
                                scalar1=b4p_s[:, 0:1], scalar2=0.0,
                                op0=ADD, op1=MAX)
                        else:
                            nc.scalar.activation(r5[:, :W], ps4[:, :W], RELU,
                                                 bias=b4p_s[:, 0:1])
                        nc.tensor.matmul(ps5[32 * j:32 * j + 3, :W], w5_s[:],
                                         r5[:, :W], start=True, stop=True,
                                         tile_position=(0, 32 * j))
                    s_t = wp.tile([99, NODE_W], F32, tag="s")
                    nc.scalar.activation(s_t[:], ps5[:], TANH,
                                         bias=b5pk_s[:, 0:1])
                    gc = g * NODE_W
                    nc.vector.scalar_tensor_tensor(
                        out=outpk_s[:, gc:gc + NODE_W], in0=s_t[:],
                        scalar=0.1, in1=pospk_s[:, gc:gc + NODE_W],
                        op0=MULT, op1=ADD)
                    # stream each group's slab out as soon as it's ready
                    nc.sync.dma_start(out_d[:, gc:gc + NODE_W],
                                      outpk_s[:, gc:gc + NODE_W])
    nc.compile()
    return nc


def make_inputs(x, pos, w1, b1, w2, b2, w3, b3, w4, b4, w5, b5,
                src, dst, sched):
    n_nodes = x.shape[0]
    E = src.shape[0]
    NC, L, d_max = sched['NC'], sched['L'], sched['d_max']
    N_r, rank_off = sched['N_r'], sched['rank_off']
    nodes_sorted = sched['nodes_sorted']
    n_groups = sched['n_groups']
    GPC = n_groups * NODE_W

    order = np.argsort(dst, kind='stable')
    src_sorted = src[order]
    deg = np.bincount(dst, minlength=n_nodes)
    starts = np.zeros(n_nodes + 1, np.int64)
    np.cumsum(deg, out=starts[1:])

    # msg @ w1 = [xi ; xj-xi] @ w1 = [xi ; xj] @ [[w1a-w1b]; [w1b]]
    w1a, w1b = w1[:3], w1[3:]
    w1m = np.zeros((128, 128), np.float32)
    w1m[:6] = np.vstack([w1a - w1b, w1b])
    w1m = w1m.astype(BF)
    b4p = (b3 @ w4 + b4).astype(np.float32).reshape(128, 1)   # fold b3
    b5pk = np.zeros((99, 1), np.float32)
    for j in range(GROUP):
        b5pk[32 * j:32 * j + 3, 0] = b5

    common = dict(
        w1m=w1m, w2=w2.astype(BF), w34=(w3 @ w4).astype(BF),
        w5=w5.astype(BF), b1=b1.reshape(128, 1).astype(np.float32),
        b2=b2.reshape(128, 1).astype(np.float32), b4p=b4p, b5pk=b5pk,
        agg0=np.full((128, NC), -1e30, BF))

    slot_pos = np.zeros(L, np.int64)
    for r in range(d_max):
        w = int(N_r[r])
        o = int(rank_off[r])
        slot_pos[o:o + w] = np.arange(w)

    in_maps = []
    for c in range(N_CORES):
        loc_nodes = nodes_sorted[c::N_CORES]
        loc_deg = deg[loc_nodes]
        loc_start = starts[loc_nodes]
        slot_src = np.zeros(L, np.int64)
        for r in range(d_max):
            w = int(N_r[r])
            o = int(rank_off[r])
            has = loc_deg[:w] > r
            # pad slots duplicate the node's first edge (max-idempotent);
            # deg-0 nodes gather garbage and are patched on the host
            idx = np.where(has, loc_start[:w] + r, loc_start[:w])
            np.minimum(idx, E - 1, out=idx)
            slot_src[o:o + w] = src_sorted[idx]
        xi_loc = x[loc_nodes]
        n_chunks = -(-L // CHUNK)
        Lp = n_chunks * CHUNK
        xs_flat = np.zeros((6, Lp), BF)
        xs_flat[0:3, :L] = xi_loc[slot_pos].T.astype(BF)
        xs_flat[3:6, :L] = x[slot_src].T.astype(BF)
        # chunk-grouped layout: [n_chunks, 6, CHUNK] -> [6*n_chunks, CHUNK]
        xs = np.ascontiguousarray(
            xs_flat.reshape(6, n_chunks, CHUNK).transpose(1, 0, 2)
        ).reshape(6 * n_chunks, CHUNK)
        # pack pos tiles 4-per-group into partition strips 32j..32j+2
        pos_t = np.zeros((3, n_groups * GROUP * NODE_W), np.float32)
        pos_t[:, :NC] = pos[loc_nodes].T
        ptiles = pos_t.reshape(3, n_groups * GROUP, NODE_W)
        pospk = np.zeros((99, n_groups, NODE_W), np.float32)
        for j in range(GROUP):
            pospk[32 * j:32 * j + 3] = ptiles[:, j::GROUP, :]
        in_maps.append(dict(xs=xs, pospk=pospk.reshape(99, GPC), **common))
    return in_maps


def unpack_outputs(results, sched, pos, deg, w3, b3, w4, b4, w5, b5):
    NC = sched['NC']
    nodes_sorted = sched['nodes_sorted']
    n_groups = sched['n_groups']
    n = len(nodes_sorted)
    out_full = np.zeros((n, 3), np.float32)
    for c in range(N_CORES):
        outpk = results[c]['outpk'].reshape(99, n_groups, NODE_W)
        tiles = np.zeros((3, n_groups * GROUP, NODE_W), np.float32)
        for j in range(GROUP):
            tiles[:, j::GROUP, :] = outpk[32 * j:32 * j + 3]
        out_t = tiles.reshape(3, -1)[:, :NC]
        out_full[nodes_sorted[c::N_CORES]] = out_t.T
    deg0 = deg == 0
    if deg0.any():
        # closed form for isolated nodes: agg = 0 -> enc = b3
        enc0 = b3
        dec0 = np.maximum(enc0 @ w4 + b4, 0.0) @ w5 + b5
        out_full[deg0] = pos[deg0] + 0.1 * np.tanh(dec0)
    return out_full


def run(inputs, trace=False, tmpdir=None):
    x = np.asarray(inputs['x'], np.float32)
    pos = np.asarray(inputs['pos'], np.float32)
    ei = np.asarray(inputs['edge_index'])
    src = ei[0].astype(np.int64)
    dst = ei[1].astype(np.int64)
    deg = np.bincount(dst, minlength=x.shape[0])
    sched = make_schedule(deg, x.shape[0])
    nc = build_nc(sched)
    args = [np.asarray(inputs[k], np.float32) for k in
            ('w1', 'b1', 'w2', 'b2', 'w3', 'b3', 'w4', 'b4', 'w5', 'b5')]
    in_maps = make_inputs(x, pos, *args, src, dst, sched)
    res = bass_utils.run_bass_kernel_spmd(
        nc, in_maps, core_ids=list(range(N_CORES)), trace=trace, tmpdir=tmpdir)
    w3_, b3_, w4_, b4_, w5_, b5_ = args[4:]
    out = unpack_outputs(res.results, sched, pos, deg,
                         w3_, b3_, w4_, b4_, w5_, b5_)
    return out, res


def kernel(**inputs):
    out, _ = run(inputs, trace=False)
    return out



# revision 56
# speedup vs baseline: 1.1959x; 1.0204x over previous
"""MeshUpdateNet (EdgeConv message passing + MLP decoder) on 8 Trainium2
NeuronCores via Bass/Tile.

Strategy (chosen over the edge-shard + all-reduce-max hint: sharding by
destination node needs no collectives at all):

  - Nodes are sharded by destination: sort nodes by degree (desc) and deal
    them round-robin to the 8 cores. Each core owns NC = N/8 nodes and all
    edges pointing at them (~E/8 per core, balanced), and its local node
    list is degree-sorted.
  - Edges are laid out rank-major: rank r holds the r-th edge of every
    local node with deg > r. Because nodes are degree-sorted, rank r's
    slots form a prefix [0, N_r) of the local node axis, so segment-max
    becomes a sequence of elementwise max ops over aligned prefixes - no
    scatter, no segmented reduce.
  - The host pre-gathers the per-slot features [xi ; xj] into a [6, L]
    bf16 stream per core (this is the sharding step: replicate+permute of
    x). The round-robin deal makes the rank widths common across cores
    (+-1, padded by duplicating an existing edge of the node - max is
    idempotent so duplicates are free), so one SPMD program serves all 8.
  - Device per core:  h1 = relu(w1m^T s + b1)  (PE K=6-pad-128 matmul + ACT)
                      h2 = w2^T h1             (PE K=128 matmul)
                      agg = max(agg, h2)       (DVE tensor_tensor, psum in)
    The stream is bound by the two PSUM evacuations (ACT relu-pass and
    DVE max-pass, both 1 elem/lane/cycle on TRN2 since matmul PSUM
    output must be fp32); measured DVE occupancy is ~99% in steady
    state, i.e. the stream runs at the hardware floor.
    Dense tail in 512-node tiles with w34 = w3@w4 folded on the host
    (no nonlinearity between them) and b3 folded into b4':
      r3 = relu(agg + b2) (DVE) -> r5 = relu(w34^T r3 + b4')
      (ACT/DVE alternating) -> dec = w5^T r5 packed 4 tiles per PSUM
      bank via tile_position col groups -> tanh (+b5) ->
      out = pos + 0.1*tanh (DVE scalar_tensor_tensor), all feature-major.
  - Scheduling: chunk DMAs are issued just-in-time inside the tile loop
    (an upfront prefetch chain serialized on Sync and stalled all
    engines ~82us); buffer zero-fills are placed per-engine so the
    in-order Vector queue stays clear of the stream's TT ops; tail
    constants are fetched mid-stream; each output group is DMA'd as
    soon as it completes; agg (bf16) is initialised to -1e30 via DMA
    from a host constant. Nodes with no edges are patched on the host
    with the closed-form constant output.
"""
import sys

sys.path.insert(0, '/opt/trn_rl_repo')

import numpy as np
import ml_dtypes

import concourse.bass as bass
import concourse.tile as tile
from concourse import bacc, mybir
from concourse import bass_utils

F32 = mybir.dt.float32
BF16 = mybir.dt.bfloat16
BF = ml_dtypes.bfloat16

N_CORES = 8
TILE_W = 1024      # edge tile width (2 psum banks)
MM_W = 512         # max matmul moving free dim
CHUNK = 8192       # stream DMA chunk (cols)
NODE_W = 512       # tail node-tile width
GROUP = 4          # node tiles packed per psum group in the tail
WARMUP_MM = 14     # gapless matmul chain to ramp the PE p-state


def make_schedule(deg, n_nodes):
    """Common (all-cores) edge/tail tiling from the global degree array."""
    nodes_sorted = np.argsort(-deg, kind='stable')
    deg_sorted = deg[nodes_sorted]
    d_max = int(deg_sorted[0]) if len(deg_sorted) else 0
    M = np.searchsorted(-deg_sorted, -(np.arange(d_max) + 1), side='right')
    NC = n_nodes // N_CORES
    N_r = -(-M // N_CORES)              # ceil(M_r/8): common rank width
    T_r = -(-N_r // TILE_W)
    L = int((T_r * TILE_W).sum())
    rank_off = np.zeros(d_max + 1, np.int64)
    np.cumsum(T_r * TILE_W, out=rank_off[1:])
    etiles = []
    for r in range(d_max):
        w_left = int(N_r[r])
        for t in range(int(T_r[r])):
            w = min(TILE_W, w_left - t * TILE_W)
            etiles.append((int(rank_off[r]) + t * TILE_W, t * TILE_W, w))
    n_ntiles = -(-NC // NODE_W)
    n_groups = -(-n_ntiles // GROUP)
    return dict(nodes_sorted=nodes_sorted, deg_sorted=deg_sorted, d_max=d_max,
                NC=NC, N_r=N_r, T_r=T_r, L=L, rank_off=rank_off, etiles=etiles,
                n_ntiles=n_ntiles, n_groups=n_groups)


def build_nc(sched):
    NC, L = sched['NC'], sched['L']
    etiles = sched['etiles']
    n_ntiles, n_groups = sched['n_ntiles'], sched['n_groups']
    GPC = n_groups * NODE_W
    n_chunks = -(-L // CHUNK)

    nc = bacc.Bacc("TRN2", target_bir_lowering=False, debug=False,
                   enable_asserts=False, num_devices=N_CORES)

    # chunk-grouped stream layout: chunk ci's 6 rows live at rows
    # [6ci, 6ci+6) and are contiguous in DRAM (better DMA locality)
    xs_d = nc.dram_tensor("xs", [6 * n_chunks, CHUNK], BF16,
                          kind="ExternalInput").ap()
    pospk_d = nc.dram_tensor("pospk", [99, GPC], F32, kind="ExternalInput").ap()
    w1m_d = nc.dram_tensor("w1m", [128, 128], BF16, kind="ExternalInput").ap()
    w2_d = nc.dram_tensor("w2", [128, 128], BF16, kind="ExternalInput").ap()
    # w34 = w3 @ w4 folded on the host (no nonlinearity between them)
    w34_d = nc.dram_tensor("w34", [128, 128], BF16, kind="ExternalInput").ap()
    w5_d = nc.dram_tensor("w5", [128, 3], BF16, kind="ExternalInput").ap()
    b1_d = nc.dram_tensor("b1", [128, 1], F32, kind="ExternalInput").ap()
    agg0_d = nc.dram_tensor("agg0", [128, NC], BF16, kind="ExternalInput").ap()
    b2_d = nc.dram_tensor("b2", [128, 1], F32, kind="ExternalInput").ap()
    b4p_d = nc.dram_tensor("b4p", [128, 1], F32, kind="ExternalInput").ap()
    b5pk_d = nc.dram_tensor("b5pk", [99, 1], F32, kind="ExternalInput").ap()
    out_d = nc.dram_tensor("outpk", [99, GPC], F32, kind="ExternalOutput").ap()

    RELU = mybir.ActivationFunctionType.Relu
    TANH = mybir.ActivationFunctionType.Tanh
    COPY = mybir.ActivationFunctionType.Copy
    ADD = mybir.AluOpType.add
    MAX = mybir.AluOpType.max
    MULT = mybir.AluOpType.mult

    NBUF = 4
    LOOKAHEAD = 2
    r0_end = int(sched['rank_off'][1])

    with tile.TileContext(nc) as tc:
        with (
            tc.tile_pool(name="const", bufs=1) as cp,
            tc.tile_pool(name="aggp", bufs=1) as aggp,
            tc.tile_pool(name="stream", bufs=1) as sp,
            tc.tile_pool(name="work", bufs=4) as wp,
        ):
            # constants needed early (first on the Sync queue)
            w2_s = cp.tile([128, 128], BF16)
            nc.sync.dma_start(w2_s[:], w2_d[:])
            w1m_s = cp.tile([128, 128], BF16)
            nc.sync.dma_start(w1m_s[:], w1m_d[:])
            b1_s = cp.tile([128, 1], F32)
            nc.sync.dma_start(b1_s[:], b1_d[:])

            warm_rhs = wp.tile([128, 512], BF16, tag="warmrhs")
            nc.vector.memset(warm_rhs[:], 0.0)

            # Stream buffers: rows 0-5 carry the DMA'd [xi;xj] stream;
            # rows 6-127 are zeroed once and never rewritten, so mm1
            # contracts over K=128 with a zero-padded w1m. buf0 on
            # Vector in quarters (fast path for the first chunk); the
            # rest on GpSimd so the Vector queue stays clear for the
            # stream's in-order TT ops.
            ch_bufs = []
            for bi in range(NBUF):
                chb = sp.tile([128, CHUNK], BF16, tag=f"xs{bi}",
                              name=f"chb{bi}")
                if bi == 0:
                    q = CHUNK // 4
                    for k in range(4):
                        nc.vector.memset(chb[:, k * q:(k + 1) * q], 0.0)
                else:
                    nc.gpsimd.memset(chb[:], 0.0)
                ch_bufs.append(chb)

            # just-in-time chunk DMA issuance (keeps the Sync queue free
            # of upfront cross-buffer waits that would gate the stream)
            issued = [-1]

            def ensure_chunk(ci):
                tgt = min(ci + LOOKAHEAD, n_chunks - 1)
                while issued[0] < tgt:
                    issued[0] += 1
                    c = issued[0]
                    ch = ch_bufs[c % NBUF]
                    # finer splits early: chunk 0's col-quarters pair
                    # with its memset quarters so the first tiles start
                    # as soon as possible
                    ncol = 4 if c == 0 else 2
                    nrow = 2 if c <= 2 else 1
                    w = CHUNK // ncol
                    for k in range(ncol):
                        for (r0, r1) in ([(0, 3), (3, 6)] if nrow == 2
                                         else [(0, 6)]):
                            nc.sync.dma_start(
                                ch[r0:r1, k * w:(k + 1) * w],
                                xs_d[6 * c + r0:6 * c + r1,
                                     k * w:(k + 1) * w])

            ensure_chunk(0)
            # agg init streamed from a host constant, in slices so the
            # first stream tiles only wait on the slice they touch
            agg = aggp.tile([128, NC], BF16)
            a3 = NC // 3
            nc.sync.dma_start(agg[:, :a3], agg0_d[:, :a3])
            nc.sync.dma_start(agg[:, a3:2 * a3], agg0_d[:, a3:2 * a3])
            nc.sync.dma_start(agg[:, 2 * a3:], agg0_d[:, 2 * a3:])

            # PE warm-up: gapless back-to-back matmul chain in its own
            # psum scope (4 slots so slot-release never stalls the chain);
            # the p-state ramp needs >3us of uninterrupted PE execution.
            # Runs while the first chunk DMAs land.
            with tc.tile_pool(name="psW", bufs=4, space="PSUM") as pW:
                for i in range(WARMUP_MM):
                    wps = pW.tile([128, 512], F32, tag="warm")
                    nc.tensor.matmul(wps[:], w2_s[:], warm_rhs[:],
                                     start=True, stop=True)

            with (
                tc.tile_pool(name="psA", bufs=2, space="PSUM") as pA,
                tc.tile_pool(name="psB", bufs=2, space="PSUM") as pB,
            ):
                # tail constants: issued mid-stream (Sync queue is sparse
                # there) so they're resident well before the tail starts
                tail_tiles = {}

                def issue_tail_consts():
                    w34_s = cp.tile([128, 128], BF16)
                    nc.sync.dma_start(w34_s[:], w34_d[:])
                    w5_s = cp.tile([128, 3], BF16)
                    nc.sync.dma_start(w5_s[:], w5_d[:])
                    b2_s = cp.tile([128, 1], F32)
                    nc.sync.dma_start(b2_s[:], b2_d[:])
                    b4p_s = cp.tile([128, 1], F32)
                    nc.sync.dma_start(b4p_s[:], b4p_d[:])
                    b5pk_s = cp.tile([99, 1], F32)
                    nc.sync.dma_start(b5pk_s[:], b5pk_d[:])
                    pospk_s = cp.tile([99, GPC], F32)
                    nc.sync.dma_start(pospk_s[:], pospk_d[:])
                    tail_tiles.update(w34_s=w34_s, w5_s=w5_s, b2_s=b2_s,
                                      b4p_s=b4p_s, b5pk_s=b5pk_s,
                                      pospk_s=pospk_s)

                for ti, (so, c0, W) in enumerate(etiles):
                    if ti == 120:
                        issue_tail_consts()
                    ci, off = so // CHUNK, so % CHUNK
                    ensure_chunk(ci)
                    ch = ch_bufs[ci % NBUF]
                    ps1 = pA.tile([128, TILE_W], F32, tag="p1")
                    for h in range(0, W, MM_W):
                        w = min(MM_W, W - h)
                        nc.tensor.matmul(ps1[:, h:h + w], w1m_s[:],
                                         ch[:, off + h: off + h + w],
                                         start=True, stop=True)
                    h1 = wp.tile([128, TILE_W], BF16, tag="h1")
                    nc.scalar.activation(h1[:, :W], ps1[:, :W], RELU,
                                         bias=b1_s[:, 0:1])
                    ps2 = pB.tile([128, TILE_W], F32, tag="p2")
                    for h in range(0, W, MM_W):
                        w = min(MM_W, W - h)
                        nc.tensor.matmul(ps2[:, h:h + w], w2_s[:],
                                         h1[:, h:h + w], start=True, stop=True)
                    nc.vector.tensor_tensor(
                        out=agg[:, c0:c0 + W], in0=ps2[:, :W],
                        in1=agg[:, c0:c0 + W], op=MAX)

            w34_s = tail_tiles['w34_s']
            w5_s = tail_tiles['w5_s']
            b2_s = tail_tiles['b2_s']
            b4p_s = tail_tiles['b4p_s']
            b5pk_s = tail_tiles['b5pk_s']
            pospk_s = tail_tiles['pospk_s']
            outpk_s = cp.tile([99, GPC], F32)

            with (
                tc.tile_pool(name="psT", bufs=4, space="PSUM") as pT,
                tc.tile_pool(name="psG", bufs=2, space="PSUM") as pG,
            ):
                for g in range(n_groups):
                    ps5 = pG.tile([99, NODE_W], F32, tag="p5")
                    nc.vector.memset(ps5[:], 0.0)
                    for j in range(GROUP):
                        t = g * GROUP + j
                        if t >= n_ntiles:
                            break
                        c0 = t * NODE_W
                        W = min(NODE_W, NC - c0)
                        # r3 = relu(agg + b2) -> bf16, on DVE (4x mode)
                        r3 = wp.tile([128, NODE_W], BF16, tag="r3")
                        nc.vector.tensor_scalar(
                            out=r3[:, :W], in0=agg[:, c0:c0 + W],
                            scalar1=b2_s[:, 0:1], scalar2=0.0,
                            op0=ADD, op1=MAX)
                        ps4 = pT.tile([128, NODE_W], F32, tag="p4")
                        nc.tensor.matmul(ps4[:, :W], w34_s[:], r3[:, :W],
                                         start=True, stop=True)
                        r5 = wp.tile([128, NODE_W], BF16, tag="r5")
                        if j == 1:
                            nc.vector.tensor_scalar(
                                out=r5[:, :W], in0=ps4[:, :W],
                                scalar1=b4p_s[:, 0:1], scalar2=0.0,
                                op0=ADD, op1=MAX)
                        else:
                            nc.scalar.activation(r5[:, :W], ps4[:, :W], RELU,
                                                 bias=b4p_s[:, 0:1])
                        nc.tensor.matmul(ps5[32 * j:32 * j + 3, :W], w5_s[:],
                                         r5[:, :W], start=True, stop=True,
                                         tile_position=(0, 32 * j))
                    s_t = wp.tile([99, NODE_W], F32, tag="s")
                    nc.scalar.activation(s_t[:], ps5[:], TANH,
                                         bias=b5pk_s[:, 0:1])
                    gc = g * NODE_W
                    nc.vector.scalar_tensor_tensor(
                        out=outpk_s[:, gc:gc + NODE_W], in0=s_t[:],
                        scalar=0.1, in1=pospk_s[:, gc:gc + NODE_W],
                        op0=MULT, op1=ADD)
                    # stream each group's slab out as soon as it's ready
                    nc.sync.dma_start(out_d[:, gc:gc + NODE_W],
                                      outpk_s[:, gc:gc + NODE_W])
    nc.compile()
    return nc


def make_inputs(x, pos, w1, b1, w2, b2, w3, b3, w4, b4, w5, b5,
                src, dst, sched):
    n_nodes = x.shape[0]
    E = src.shape[0]
    NC, L, d_max = sched['NC'], sched['L'], sched['d_max']
    N_r, rank_off = sched['N_r'], sched['rank_off']
    nodes_sorted = sched['nodes_sorted']
    n_groups = sched['n_groups']
    GPC = n_groups * NODE_W

    order = np.argsort(dst, kind='stable')
    src_sorted = src[order]
    deg = np.bincount(dst, minlength=n_nodes)
    starts = np.zeros(n_nodes + 1, np.int64)
    np.cumsum(deg, out=starts[1:])

    # msg @ w1 = [xi ; xj-xi] @ w1 = [xi ; xj] @ [[w1a-w1b]; [w1b]]
    w1a, w1b = w1[:3], w1[3:]
    w1m = np.zeros((128, 128), np.float32)
    w1m[:6] = np.vstack([w1a - w1b, w1b])
    w1m = w1m.astype(BF)
    b4p = (b3 @ w4 + b4).astype(np.float32).reshape(128, 1)   # fold b3
    b5pk = np.zeros((99, 1), np.float32)
    for j in range(GROUP):
        b5pk[32 * j:32 * j + 3, 0] = b5

    common = dict(
        w1m=w1m, w2=w2.astype(BF), w34=(w3 @ w4).astype(BF),
        w5=w5.astype(BF), b1=b1.reshape(128, 1).astype(np.float32),
        b2=b2.reshape(128, 1).astype(np.float32), b4p=b4p, b5pk=b5pk,
        agg0=np.full((128, NC), -1e30, BF))

    slot_pos = np.zeros(L, np.int64)
    for r in range(d_max):
        w = int(N_r[r])
        o = int(rank_off[r])
        slot_pos[o:o + w] = np.arange(w)

    in_maps = []
    for c in range(N_CORES):
        loc_nodes = nodes_sorted[c::N_CORES]
        loc_deg = deg[loc_nodes]
        loc_start = starts[loc_nodes]
        slot_src = np.zeros(L, np.int64)
        for r in range(d_max):
            w = int(N_r[r])
            o = int(rank_off[r])
            has = loc_deg[:w] > r
            # pad slots duplicate the node's first edge (max-idempotent);
            # deg-0 nodes gather garbage and are patched on the host
            idx = np.where(has, loc_start[:w] + r, loc_start[:w])
            np.minimum(idx, E - 1, out=idx)
            slot_src[o:o + w] = src_sorted[idx]
        xi_loc = x[loc_nodes]
        n_chunks = -(-L // CHUNK)
        Lp = n_chunks * CHUNK
        xs_flat = np.zeros((6, Lp), BF)
        xs_flat[0:3, :L] = xi_loc[slot_pos].T.astype(BF)
        xs_flat[3:6, :L] = x[slot_src].T.astype(BF)
        # chunk-grouped layout: [n_chunks, 6, CHUNK] -> [6*n_chunks, CHUNK]
        xs = np.ascontiguousarray(
            xs_flat.reshape(6, n_chunks, CHUNK).transpose(1, 0, 2)
        ).reshape(6 * n_chunks, CHUNK)
        # pack pos tiles 4-per-group into partition strips 32j..32j+2
        pos_t = np.zeros((3, n_groups * GROUP * NODE_W), np.float32)
        pos_t[:, :NC] = pos[loc_nodes].T
        ptiles = pos_t.reshape(3, n_groups * GROUP, NODE_W)
        pospk = np.zeros((99, n_groups, NODE_W), np.float32)
        for j in range(GROUP):
            pospk[32 * j:32 * j + 3] = ptiles[:, j::GROUP, :]
        in_maps.append(dict(xs=xs, pospk=pospk.reshape(99, GPC), **common))
    return in_maps


def unpack_outputs(results, sched, pos, deg, w3, b3, w4, b4, w5, b5):
    NC = sched['NC']
    nodes_sorted = sched['nodes_sorted']
    n_groups = sched['n_groups']
    n = len(nodes_sorted)
    out_full = np.zeros((n, 3), np.float32)
    for c in range(N_CORES):
        outpk = results[c]['outpk'].reshape(99, n_groups, NODE_W)
        tiles = np.zeros((3, n_groups * GROUP, NODE_W), np.float32)
        for j in range(GROUP):
            tiles[:, j::GROUP, :] = outpk[32 * j:32 * j + 3]
        out_t = tiles.reshape(3, -1)[:, :NC]
        out_full[nodes_sorted[c::N_CORES]] = out_t.T
    deg0 = deg == 0
    if deg0.any():
        # closed form for isolated nodes: agg = 0 -> enc = b3
        enc0 = b3
        dec0 = np.maximum(enc0 @ w4 + b4, 0.0) @ w5 + b5
        out_full[deg0] = pos[deg0] + 0.1 * np.tanh(dec0)
    return out_full


def run(inputs, trace=False, tmpdir=None):
    x = np.asarray(inputs['x'], np.float32)
    pos = np.asarray(inputs['pos'], np.float32)
    ei = np.asarray(inputs['edge_index'])
    src = ei[0].astype(np.int64)
    dst = ei[1].astype(np.int64)
    deg = np.bincount(dst, minlength=x.shape[0])
    sched = make_schedule(deg, x.shape[0])
    nc = build_nc(sched)
    args = [np.asarray(inputs[k], np.float32) for k in
            ('w1', 'b1', 'w2', 'b2', 'w3', 'b3', 'w4', 'b4', 'w5', 'b5')]
    in_maps = make_inputs(x, pos, *args, src, dst, sched)
    res = bass_utils.run_bass_kernel_spmd(
        nc, in_maps, core_ids=list(range(N_CORES)), trace=trace, tmpdir=tmpdir)
    w3_, b3_, w4_, b4_, w5_, b5_ = args[4:]
    out = unpack_outputs(res.results, sched, pos, deg,
                         w3_, b3_, w4_, b4_, w5_, b5_)
    return out, res


def kernel(**inputs):
    out, _ = run(inputs, trace=False)
    return out

